# revision 3
# baseline (speedup 1.0000x reference)
"""Multi-head attention on 8 Trainium2 NeuronCores.

Problem shape: x[4, 2048, 1024], H=16 heads, Dh=64, fp32.
Sharding: core c handles batch b = c//2 and heads 8*(c%2) .. 8*(c%2)+8.
Each core computes its 8 heads' attention + the partial W_O contraction
for its batch; the host sums the two half-head partials per batch and
adds b_O.  No collectives needed.

Device-side layout (per core, all host-pre-transposed so the kernel
never transposes its inputs):
  xT   [1024, 2048]  = x[b].T                      (d on rows)
  wqT/wkT/wvT [1024, 512] = W[heads].reshape(512,1024).T   ([d, (h,k)])
  woT  [512, 1024]   = W_O[heads].transpose(0,2,1).reshape ([(h,k), d])
  bq/bk/bv [128, 4]  per-partition bias layout (col m = (h,k) m*128..)
Output: out [2048, 1024] partial (pre-b_O) for this core's batch.
"""

import numpy as np
from contextlib import ExitStack

import concourse.bass as bass
import concourse.mybir as mybir
import concourse.tile as tile
from concourse import bacc
from concourse.bass_utils import run_bass_kernel_spmd
from concourse.masks import make_identity

F32 = mybir.dt.float32
AF = mybir.ActivationFunctionType

T = 2048          # tokens
D = 1024          # d_model
HK = 512          # 8 local heads x 64
NH = 8            # local heads
DH = 64           # head dim
NDT = 8           # d-tiles of 128
NTT = 16          # t-tiles of 128
NMT = 4           # (h,k) m-tiles of 128
NQC = 4           # q-chunks of 512
NST = 16          # s-tiles of 128
VW = NH * (DH + 1)  # V_aug width: 8 heads x (64 + ones col)

# dtype knobs (bitcast for matmul inputs only; storage dtype separate)
MM_CAST = None    # e.g. mybir.dt.float32r to run PE in fp32r


def _mc(ap):
    """Optionally bitcast a matmul operand."""
    if MM_CAST is None:
        return ap
    return ap.bitcast(MM_CAST)


def build():
    nc = bacc.Bacc("TRN2", target_bir_lowering=False, debug=False)

    xT_d = nc.dram_tensor("xT", [D, T], F32, kind="ExternalInput").ap()
    wq_d = nc.dram_tensor("wqT", [D, HK], F32, kind="ExternalInput").ap()
    wk_d = nc.dram_tensor("wkT", [D, HK], F32, kind="ExternalInput").ap()
    wv_d = nc.dram_tensor("wvT", [D, HK], F32, kind="ExternalInput").ap()
    wo_d = nc.dram_tensor("woT", [HK, D], F32, kind="ExternalInput").ap()
    bq_d = nc.dram_tensor("bq", [128, 4], F32, kind="ExternalInput").ap()
    bk_d = nc.dram_tensor("bk", [128, 4], F32, kind="ExternalInput").ap()
    bv_d = nc.dram_tensor("bv", [128, 4], F32, kind="ExternalInput").ap()
    out_d = nc.dram_tensor("out", [T, D], F32, kind="ExternalOutput").ap()

    with tile.TileContext(nc) as tc, ExitStack() as ctx:
        const = ctx.enter_context(tc.tile_pool(name="const", bufs=1))
        ident = const.tile([128, 128], F32, tag="ident", name="ident")
        make_identity(nc, ident[:])
        bq_sb = const.tile([128, 4], F32, tag="bq", name="bq")
        bk_sb = const.tile([128, 4], F32, tag="bk", name="bk")
        bv_sb = const.tile([128, 4], F32, tag="bv", name="bv")
        nc.sync.dma_start(bq_sb[:], bq_d)
        nc.sync.dma_start(bk_sb[:], bk_d)
        nc.sync.dma_start(bv_sb[:], bv_d)

        persist = ctx.enter_context(tc.tile_pool(name="persist", bufs=1))
        QT = [persist.tile([128, T], F32, tag=f"qt{m}", name=f"qt{m}") for m in range(NMT)]
        KT = [persist.tile([128, T], F32, tag=f"kt{m}", name=f"kt{m}") for m in range(NMT)]
        V = [persist.tile([128, VW], F32, tag=f"v{t}", name=f"v{t}") for t in range(NTT)]

        dram = ctx.enter_context(tc.tile_pool(name="dram", bufs=1, space="DRAM"))
        OT_d = [dram.tile([128, T], F32, tag=f"otd{j}", name=f"otd{j}") for j in range(NMT)]

        # ---------------- QKV projections ----------------
        with tc.tile_pool(name="wpool", bufs=1) as wpool, \
             tc.tile_pool(name="xpool", bufs=2) as xpool, \
             tc.tile_pool(name="qkv_ps", bufs=4, space="PSUM") as qps:
            wq_sb = [wpool.tile([128, HK], F32, tag=f"wq{i}", name=f"wq{i}") for i in range(NDT)]
            wk_sb = [wpool.tile([128, HK], F32, tag=f"wk{i}", name=f"wk{i}") for i in range(NDT)]
            wv_sb = [wpool.tile([128, HK], F32, tag=f"wv{i}", name=f"wv{i}") for i in range(NDT)]
            for i in range(NDT):
                nc.sync.dma_start(wq_sb[i][:], wq_d[i * 128:(i + 1) * 128, :])
                nc.sync.dma_start(wk_sb[i][:], wk_d[i * 128:(i + 1) * 128, :])
                nc.sync.dma_start(wv_sb[i][:], wv_d[i * 128:(i + 1) * 128, :])

            for c in range(4):  # t-chunks of 512
                csl = slice(c * 512, (c + 1) * 512)
                xt = [xpool.tile([128, 512], F32, tag=f"x{i}", name=f"x{i}") for i in range(NDT)]
                for i in range(NDT):
                    nc.sync.dma_start(xt[i][:], xT_d[i * 128:(i + 1) * 128, csl])
                # Q^T and K^T m-tiles for this chunk
                for m in range(NMT):
                    msl = slice(m * 128, (m + 1) * 128)
                    ps = qps.tile([128, 512], F32, tag="ps", name="ps")
                    for i in range(NDT):
                        nc.tensor.matmul(ps[:], _mc(wq_sb[i][:, msl]), _mc(xt[i][:]),
                                         start=(i == 0), stop=(i == NDT - 1))
                    nc.scalar.activation(QT[m][:, csl], ps[:], AF.Identity,
                                         bias=bq_sb[:, m:m + 1])
                    ps = qps.tile([128, 512], F32, tag="ps", name="ps")
                    for i in range(NDT):
                        nc.tensor.matmul(ps[:], _mc(wk_sb[i][:, msl]), _mc(xt[i][:]),
                                         start=(i == 0), stop=(i == NDT - 1))
                    nc.scalar.activation(KT[m][:, csl], ps[:], AF.Identity,
                                         bias=bk_sb[:, m:m + 1])
                # V t-tiles for this chunk (natural [t, (h,k)] layout + ones col)
                for vt in range(4):
                    t_idx = c * 4 + vt
                    vsl = slice(vt * 128, (vt + 1) * 128)
                    nc.gpsimd.memset(V[t_idx][:], 1.0)
                    ps = qps.tile([128, 512], F32, tag="ps", name="ps")
                    for i in range(NDT):
                        nc.tensor.matmul(ps[:], _mc(xt[i][:, vsl]), _mc(wv_sb[i][:]),
                                         start=(i == 0), stop=(i == NDT - 1))
                    dst = V[t_idx][:].rearrange("p (h c) -> p h c", c=DH + 1)[:, :, 0:DH]
                    src = ps[:].rearrange("p (h c) -> p h c", c=DH)
                    nc.vector.tensor_copy(dst, src)

        # ---------------- attention (per head pair) ----------------
        with tc.tile_pool(name="epool", bufs=1) as epool, \
             tc.tile_pool(name="sc_ps", bufs=2, space="PSUM") as scps, \
             tc.tile_pool(name="av_ps", bufs=2, space="PSUM") as avps, \
             tc.tile_pool(name="ot_ps", bufs=2, space="PSUM") as otps, \
             tc.tile_pool(name="opool", bufs=3) as opool, \
             tc.tile_pool(name="otev", bufs=2) as otev:
            for j in range(NMT):  # head pair j: heads 2j (part 0-63), 2j+1 (64-127)
                for qc in range(NQC):
                    qsl = slice(qc * 512, (qc + 1) * 512)
                    # scores^T + exp, both heads row-packed on PE
                    exp_sb = {}
                    for st in range(NST):
                        ssl = slice(st * 128, (st + 1) * 128)
                        for hl in (0, 1):
                            psl = slice(hl * 64, (hl + 1) * 64)
                            sc = scps.tile([128, 512], F32, tag=f"sc{hl}", name=f"sc{hl}")
                            nc.tensor.matmul(sc[:], _mc(KT[j][psl, ssl]),
                                             _mc(QT[j][psl, qsl]))
                            e = epool.tile([128, 512], F32, tag=f"e{hl}_{st}", name=f"e{hl}_{st}")
                            nc.scalar.activation(e[:], sc[:], AF.Exp, scale=0.125)
                            exp_sb[(hl, st)] = e
                    # attn @ V_aug per q-subtile, normalize, transpose
                    for qt in range(4):
                        opair = opool.tile([128, 128], F32, tag="opair",
                                           name="opair")
                        for hl in (0, 1):
                            h = 2 * j + hl
                            av = avps.tile([128, 65], F32, tag="av", name="av")
                            for st in range(NST):
                                nc.tensor.matmul(
                                    av[:],
                                    _mc(exp_sb[(hl, st)][:, qt * 128:(qt + 1) * 128]),
                                    _mc(V[st][:, h * 65:h * 65 + 65]),
                                    start=(st == 0), stop=(st == NST - 1))
                            rc = opool.tile([128, 1], F32, tag="recip", name="recip")
                            nc.vector.reciprocal(rc[:], av[:, 64:65])
                            nc.vector.tensor_scalar_mul(
                                opair[:, hl * 64:(hl + 1) * 64], av[:, 0:DH], rc[:])
                        otp = otps.tile([128, 128], F32, tag="otp", name="otp")
                        nc.tensor.transpose(otp[:], opair[:], ident[:])
                        ev = otev.tile([128, 128], F32, tag="otev", name="otev")
                        nc.scalar.activation(ev[:], otp[:], AF.Identity,
                                             bias=bv_sb[:, j:j + 1])
                        tq = qc * 512 + qt * 128
                        nc.sync.dma_start(OT_d[j][:, tq:tq + 128], ev[:])

        # ---------------- output projection ----------------
        with tc.tile_pool(name="fwp", bufs=1) as fwp, \
             tc.tile_pool(name="fop", bufs=1) as fop, \
             tc.tile_pool(name="fps", bufs=3, space="PSUM") as fps, \
             tc.tile_pool(name="foutp", bufs=3) as foutp:
            wo_sb = [fwp.tile([128, D], F32, tag=f"wo{jj}", name=f"wo{jj}") for jj in range(NMT)]
            OT_sb = [fop.tile([128, T], F32, tag=f"ot{jj}", name=f"ot{jj}") for jj in range(NMT)]
            for jj in range(NMT):
                nc.sync.dma_start(wo_sb[jj][:], wo_d[jj * 128:(jj + 1) * 128, :])
                nc.sync.dma_start(OT_sb[jj][:], OT_d[jj][:])
            for t in range(NTT):
                tsl = slice(t * 128, (t + 1) * 128)
                for dc in range(2):
                    dsl = slice(dc * 512, (dc + 1) * 512)
                    ps = fps.tile([128, 512], F32, tag="fp", name="fp")
                    for jj in range(NMT):
                        nc.tensor.matmul(ps[:], _mc(OT_sb[jj][:, tsl]),
                                         _mc(wo_sb[jj][:, dsl]),
                                         start=(jj == 0), stop=(jj == NMT - 1))
                    ob = foutp.tile([128, 512], F32, tag="ob", name="ob")
                    nc.vector.tensor_copy(ob[:], ps[:])
                    nc.sync.dma_start(out_d[tsl, dsl], ob[:])

    nc.compile()
    return nc


_NC_CACHE = None


def _get_nc():
    global _NC_CACHE
    if _NC_CACHE is None:
        _NC_CACHE = build()
    return _NC_CACHE


def _prep_core(x, W_Q, b_Q, W_K, b_K, W_V, b_V, W_O, core):
    b = core // 2
    hs = slice(8 * (core % 2), 8 * (core % 2) + 8)
    f32 = np.float32

    def bias_layout(bx):
        return np.ascontiguousarray(bx[hs].reshape(4, 128).T, dtype=f32)

    return {
        "xT": np.ascontiguousarray(x[b].T, dtype=f32),
        "wqT": np.ascontiguousarray(W_Q[hs].reshape(HK, D).T, dtype=f32),
        "wkT": np.ascontiguousarray(W_K[hs].reshape(HK, D).T, dtype=f32),
        "wvT": np.ascontiguousarray(W_V[hs].reshape(HK, D).T, dtype=f32),
        "woT": np.ascontiguousarray(
            W_O[hs].transpose(0, 2, 1).reshape(HK, D), dtype=f32),
        "bq": bias_layout(b_Q),
        "bk": bias_layout(b_K),
        "bv": bias_layout(b_V),
    }


def kernel(x, W_Q, b_Q, W_K, b_K, W_V, b_V, W_O, b_O, _trace=False):
    nc = _get_nc()
    in_maps = [
        _prep_core(x, W_Q, b_Q, W_K, b_K, W_V, b_V, W_O, c) for c in range(8)
    ]
    res = run_bass_kernel_spmd(nc, in_maps, core_ids=list(range(8)),
                               trace=_trace)
    out = np.empty((4, T, D), dtype=np.float32)
    for b in range(4):
        out[b] = (res.results[2 * b]["out"] + res.results[2 * b + 1]["out"]
                  + b_O[None, :].astype(np.float32))
    if _trace:
        kernel.last_results = res
    return out


# revision 5
# speedup vs baseline: 1.5766x; 1.5766x over previous
"""Multi-head attention on 8 Trainium2 NeuronCores.

Problem shape: x[4, 2048, 1024], H=16 heads, Dh=64, fp32.
Sharding: core c handles batch b = c//2 and heads 8*(c%2) .. 8*(c%2)+8.
Each core computes its 8 heads' attention + the partial W_O contraction
for its batch; the host sums the two half-head partials per batch and
adds b_O (plus the b_V @ W_O constant row, folded host-side since
softmax rows sum to 1).  No collectives needed.

All matmuls run in float32r (fp32 storage, PE rounds to 12-bit
mantissa, 4x the fp32 rate at free-dim >= 256).  Host pre-rounds the
DRAM inputs to fp32r (RNE at 12 low mantissa bits) so DMA-loaded
operands satisfy the verifier's "rounded to FP32r" rule; on-chip
producers (ACT/DVE evictions) write float32r-typed tiles.

Device-side layout (per core, all host-pre-transposed so the kernel
never transposes anything):
  xT   [1024, 2048]  = x[b].T                                 [d, t]
  wqT/wkT/wvT [1024, 512] = W[heads].reshape(512,1024).T      [d, (h,k)]
  woT  [512, 1024]   = W_O[heads].transpose(0,2,1).reshape    [(h,k), d]
  bq/bk [128, 4]     per-partition bias layout (col m = (h,k) m*128..)
Pipeline per core:
  Q^T,K^T = W^T x^T  (+bias via ACT eviction)      [(h,k), t]
  V       = x W^T    ([t, 8*(64+1)] with a ones column per head)
  per head pair, per q-chunk: scores^T = K_h Q_h^T  (row-packed K=64
  pairs), exp on ACT (scale=1/8; scores are O(0.2), no max needed),
  O^T_unnorm/denom = V_aug^T exp^T  ([65, q], denom = row 64),
  normalize via reciprocal + K=1 broadcast matmul + DVE multiply,
  spill O^T to DRAM; finally out = O^T^T woT re-loaded per t-tile.
Output: out [2048, 1024] partial (pre-bias) for this core's batch.
"""

import numpy as np
from contextlib import ExitStack

import concourse.bass as bass
import concourse.mybir as mybir
import concourse.tile as tile
from concourse import bacc
from concourse.bass_utils import run_bass_kernel_spmd

F32 = mybir.dt.float32
F32R = mybir.dt.float32r
AF = mybir.ActivationFunctionType

T = 2048          # tokens
D = 1024          # d_model
HK = 512          # 8 local heads x 64
NH = 8            # local heads
DH = 64           # head dim
NDT = 8           # d-tiles of 128
NTT = 16          # t-tiles of 128
NMT = 4           # (h,k) m-tiles of 128
NQC = 4           # q-chunks of 512
NST = 16          # s-tiles of 128
VW = NH * (DH + 1)  # V_aug width: 8 heads x (64 + ones col)


def build():
    nc = bacc.Bacc("TRN2", target_bir_lowering=False, debug=False)

    xT_d = nc.dram_tensor("xT", [D, T], F32R, kind="ExternalInput").ap()
    wq_d = nc.dram_tensor("wqT", [D, HK], F32R, kind="ExternalInput").ap()
    wk_d = nc.dram_tensor("wkT", [D, HK], F32R, kind="ExternalInput").ap()
    wv_d = nc.dram_tensor("wvT", [D, HK], F32R, kind="ExternalInput").ap()
    wo_d = nc.dram_tensor("woT", [HK, D], F32R, kind="ExternalInput").ap()
    bq_d = nc.dram_tensor("bq", [128, 4], F32, kind="ExternalInput").ap()
    bk_d = nc.dram_tensor("bk", [128, 4], F32, kind="ExternalInput").ap()
    ones_d = nc.dram_tensor("ones", [128, DH], F32R, kind="ExternalInput").ap()
    out_d = nc.dram_tensor("out", [T, D], F32, kind="ExternalOutput").ap()

    with tile.TileContext(nc) as tc, ExitStack() as ctx:
        const = ctx.enter_context(tc.tile_pool(name="const", bufs=1))
        bq_sb = const.tile([128, 4], F32, tag="bq", name="bq")
        bk_sb = const.tile([128, 4], F32, tag="bk", name="bk")
        ones_sb = const.tile([128, DH], F32R, tag="ones", name="ones")
        nc.sync.dma_start(bq_sb[:], bq_d)
        nc.sync.dma_start(bk_sb[:], bk_d)
        nc.sync.dma_start(ones_sb[:], ones_d)

        persist = ctx.enter_context(tc.tile_pool(name="persist", bufs=1))
        QT = [persist.tile([128, T], F32R, tag=f"qt{m}", name=f"qt{m}")
              for m in range(NMT)]
        KT = [persist.tile([128, T], F32R, tag=f"kt{m}", name=f"kt{m}")
              for m in range(NMT)]
        V = [persist.tile([128, VW], F32R, tag=f"v{t}", name=f"v{t}")
             for t in range(NTT)]

        dram = ctx.enter_context(tc.tile_pool(name="dram", bufs=1, space="DRAM"))
        OT_d = [dram.tile([128, T], F32R, tag=f"otd{j}", name=f"otd{j}")
                for j in range(NMT)]

        # ---------------- QKV projections ----------------
        with tc.tile_pool(name="wpool", bufs=1) as wpool, \
             tc.tile_pool(name="xpool", bufs=2) as xpool, \
             tc.tile_pool(name="qkv_ps", bufs=4, space="PSUM") as qps:
            wq_sb = [wpool.tile([128, HK], F32R, tag=f"wq{i}", name=f"wq{i}")
                     for i in range(NDT)]
            wk_sb = [wpool.tile([128, HK], F32R, tag=f"wk{i}", name=f"wk{i}")
                     for i in range(NDT)]
            wv_sb = [wpool.tile([128, HK], F32R, tag=f"wv{i}", name=f"wv{i}")
                     for i in range(NDT)]
            for i in range(NDT):
                nc.sync.dma_start(wq_sb[i][:], wq_d[i * 128:(i + 1) * 128, :])
                nc.sync.dma_start(wk_sb[i][:], wk_d[i * 128:(i + 1) * 128, :])
                nc.sync.dma_start(wv_sb[i][:], wv_d[i * 128:(i + 1) * 128, :])

            for c in range(4):  # t-chunks of 512
                csl = slice(c * 512, (c + 1) * 512)
                xt = [xpool.tile([128, 512], F32R, tag=f"x{i}", name=f"x{i}")
                      for i in range(NDT)]
                for i in range(NDT):
                    nc.sync.dma_start(xt[i][:], xT_d[i * 128:(i + 1) * 128, csl])
                # Q^T and K^T m-tiles for this chunk
                for m in range(NMT):
                    msl = slice(m * 128, (m + 1) * 128)
                    ps = qps.tile([128, 512], F32, tag="ps", name="ps")
                    for i in range(NDT):
                        nc.tensor.matmul(ps[:], wq_sb[i][:, msl], xt[i][:],
                                         start=(i == 0), stop=(i == NDT - 1))
                    nc.scalar.activation(QT[m][:, csl], ps[:], AF.Identity,
                                         bias=bq_sb[:, m:m + 1])
                    ps = qps.tile([128, 512], F32, tag="ps", name="ps")
                    for i in range(NDT):
                        nc.tensor.matmul(ps[:], wk_sb[i][:, msl], xt[i][:],
                                         start=(i == 0), stop=(i == NDT - 1))
                    nc.scalar.activation(KT[m][:, csl], ps[:], AF.Identity,
                                         bias=bk_sb[:, m:m + 1])
                # V t-tiles for this chunk (natural [t, (h,k)] layout + ones)
                for vt in range(4):
                    t_idx = c * 4 + vt
                    vsl = slice(vt * 128, (vt + 1) * 128)
                    ps = qps.tile([128, 512], F32, tag="ps", name="ps")
                    for i in range(NDT):
                        nc.tensor.matmul(ps[:], xt[i][:, vsl], wv_sb[i][:],
                                         start=(i == 0), stop=(i == NDT - 1))
                    v3 = V[t_idx][:].rearrange("p (h c) -> p h c", c=DH + 1)
                    nc.vector.tensor_copy(
                        v3[:, :, 0:DH], ps[:].rearrange("p (h c) -> p h c", c=DH))
                    nc.vector.tensor_copy(
                        v3[:, :, DH:DH + 1],
                        ones_sb[:, 0:NH].rearrange("p (h o) -> p h o", o=1))

        # ---------------- attention (per head pair) ----------------
        # scores^T per (st,st+1) pair land in one 2-bank psum tile so exp
        # runs at N=1024; AV consumes each exp tile immediately (flash
        # style), accumulating O^T/denom in a [65, 512] psum per head.
        with tc.tile_pool(name="epool", bufs=2) as epool, \
             tc.tile_pool(name="sc_ps", bufs=1, space="PSUM") as scps, \
             tc.tile_pool(name="av_ps", bufs=1, space="PSUM") as avps, \
             tc.tile_pool(name="bc_ps", bufs=1, space="PSUM") as bcps, \
             tc.tile_pool(name="opool", bufs=3) as opool:
            for j in range(NMT):  # head pair j: heads 2j (rows 0-63), 2j+1
                for qc in range(NQC):
                    qsl = slice(qc * 512, (qc + 1) * 512)
                    avp = {}
                    for hl in (0, 1):
                        avp[hl] = avps.tile([DH + 1, 512], F32,
                                            tag=f"av{hl}", name=f"av{hl}")
                    for sp in range(NST // 2):  # s-tile pairs
                        for hl in (0, 1):
                            h = 2 * j + hl
                            psl = slice(hl * 64, (hl + 1) * 64)
                            sc = scps.tile([128, 1024], F32,
                                           tag=f"sc{hl}", name=f"sc{hl}")
                            for k in (0, 1):
                                st = 2 * sp + k
                                ssl = slice(st * 128, (st + 1) * 128)
                                nc.tensor.matmul(sc[:, k * 512:(k + 1) * 512],
                                                 KT[j][psl, ssl],
                                                 QT[j][psl, qsl])
                            e = epool.tile([128, 1024], F32R,
                                           tag=f"e{hl}", name=f"e{hl}")
                            nc.scalar.activation(e[:], sc[:], AF.Exp,
                                                 scale=0.125)
                            for k in (0, 1):
                                st = 2 * sp + k
                                nc.tensor.matmul(
                                    avp[hl][:],
                                    V[st][:, h * 65:h * 65 + 65],
                                    e[:, k * 512:(k + 1) * 512],
                                    start=(st == 0), stop=(st == NST - 1))
                    for hl in (0, 1):
                        # normalize: recip of denom row, broadcast via K=1
                        # matmul, multiply, spill to OT dram
                        rcp = opool.tile([1, 512], F32R, tag="rcp", name="rcp")
                        with nc.allow_low_precision(reason="fp32r softmax recip"):
                            nc.vector.reciprocal(rcp[:], avp[hl][DH:DH + 1, :])
                        bc = bcps.tile([DH, 512], F32, tag="bc", name="bc")
                        nc.tensor.matmul(bc[:], ones_sb[0:1, 0:DH], rcp[:])
                        bcs = opool.tile([DH, 512], F32R, tag="bcs", name="bcs")
                        nc.vector.tensor_copy(bcs[:], bc[:])
                        ot = opool.tile([DH, 512], F32R, tag="ot", name="ot")
                        nc.vector.tensor_mul(ot[:], avp[hl][0:DH, :], bcs[:])
                        nc.sync.dma_start(
                            OT_d[j][hl * 64:(hl + 1) * 64, qsl], ot[:])

        # ---------------- output projection ----------------
        with tc.tile_pool(name="fwp", bufs=1) as fwp, \
             tc.tile_pool(name="fop", bufs=1) as fop, \
             tc.tile_pool(name="fps", bufs=3, space="PSUM") as fps, \
             tc.tile_pool(name="foutp", bufs=3) as foutp:
            wo_sb = [fwp.tile([128, D], F32R, tag=f"wo{jj}", name=f"wo{jj}")
                     for jj in range(NMT)]
            OT_sb = [fop.tile([128, T], F32R, tag=f"otsb{jj}", name=f"otsb{jj}")
                     for jj in range(NMT)]
            for jj in range(NMT):
                nc.sync.dma_start(wo_sb[jj][:], wo_d[jj * 128:(jj + 1) * 128, :])
                nc.sync.dma_start(OT_sb[jj][:], OT_d[jj][:])
            for t in range(NTT):
                tsl = slice(t * 128, (t + 1) * 128)
                for dc in range(2):
                    dsl = slice(dc * 512, (dc + 1) * 512)
                    ps = fps.tile([128, 512], F32, tag="fp", name="fp")
                    for jj in range(NMT):
                        nc.tensor.matmul(ps[:], OT_sb[jj][:, tsl],
                                         wo_sb[jj][:, dsl],
                                         start=(jj == 0), stop=(jj == NMT - 1))
                    ob = foutp.tile([128, 512], F32, tag="ob", name="ob")
                    nc.vector.tensor_copy(ob[:], ps[:])
                    nc.sync.dma_start(out_d[tsl, dsl], ob[:])

    nc.compile()
    return nc


_NC_CACHE = None


def _get_nc():
    global _NC_CACHE
    if _NC_CACHE is None:
        _NC_CACHE = build()
    return _NC_CACHE


def _round_f32r(x):
    b = np.ascontiguousarray(x, dtype=np.float32).view(np.uint32)
    r = (b + 0x7FF + ((b >> 12) & 1)) & np.uint32(0xFFFFF000)
    return r.view(np.float32)


def _prep_core(x, W_Q, b_Q, W_K, b_K, W_V, b_V, W_O, core):
    b = core // 2
    hs = slice(8 * (core % 2), 8 * (core % 2) + 8)
    f32 = np.float32

    def bias_layout(bx):
        return np.ascontiguousarray(bx[hs].reshape(4, 128).T, dtype=f32)

    return {
        "xT": _round_f32r(x[b].T),
        "wqT": _round_f32r(W_Q[hs].reshape(HK, D).T),
        "wkT": _round_f32r(W_K[hs].reshape(HK, D).T),
        "wvT": _round_f32r(W_V[hs].reshape(HK, D).T),
        "woT": _round_f32r(W_O[hs].transpose(0, 2, 1).reshape(HK, D)),
        "bq": bias_layout(b_Q),
        "bk": bias_layout(b_K),
        "ones": np.ones((128, DH), dtype=f32),
    }


def kernel(x, W_Q, b_Q, W_K, b_K, W_V, b_V, W_O, b_O, _trace=False):
    nc = _get_nc()
    in_maps = [
        _prep_core(x, W_Q, b_Q, W_K, b_K, W_V, b_V, W_O, c) for c in range(8)
    ]
    res = run_bass_kernel_spmd(nc, in_maps, core_ids=list(range(8)),
                               trace=_trace)
    out = np.empty((4, T, D), dtype=np.float32)
    for b in range(4):
        # b_V enters additively after softmax (rows sum to 1): fold
        # b_V @ W_O per half-head shard into the host-side bias.
        acc = res.results[2 * b]["out"].astype(np.float32).copy()
        acc += res.results[2 * b + 1]["out"]
        bias = b_O.astype(np.float64).copy()
        for c in (2 * b, 2 * b + 1):
            hs = slice(8 * (c % 2), 8 * (c % 2) + 8)
            bias += np.einsum("hk,hdk->d", b_V[hs].astype(np.float64),
                              W_O[hs].astype(np.float64))
        out[b] = acc + bias.astype(np.float32)[None, :]
    if _trace:
        kernel.last_results = res
    return out


# revision 7
# speedup vs baseline: 2.0239x; 1.2837x over previous
"""Multi-head attention on 8 Trainium2 NeuronCores.

Problem shape: x[4, 2048, 1024], H=16 heads, Dh=64, fp32.
Sharding: core c handles batch b = c//2 and heads 8*(c%2) .. 8*(c%2)+8.
Each core computes its 8 heads' attention + the partial W_O contraction
for its batch; the host sums the two half-head partials per batch and
adds b_O (plus the b_V @ W_O constant row, folded host-side since
softmax rows sum to 1).  No collectives needed.

All matmuls run in float32r (fp32 storage, PE rounds to 12-bit
mantissa, 4x the fp32 rate at free-dim >= 256).  Host pre-rounds the
DRAM inputs to fp32r (RNE at 12 low mantissa bits) so DMA-loaded
operands satisfy the verifier's "rounded to FP32r" rule; on-chip
producers (ACT/DVE evictions) write float32r-typed tiles.

Device-side layout (per core, all host-pre-transposed so the kernel
never transposes anything):
  xT   [1024, 2048]  = x[b].T                                 [d, t]
  wqT/wkT/wvT [1024, 512] = W[heads].reshape(512,1024).T      [d, (h,k)]
  woT  [512, 1024]   = W_O[heads].transpose(0,2,1).reshape    [(h,k), d]
  bq/bk [128, 4]     per-partition bias layout (col m = (h,k) m*128..)
Pipeline per core:
  Q^T,K^T = W^T x^T  (+bias via ACT eviction)      [(h,k), t]
  V       = x W^T    ([t, 8*(64+1)] with a ones column per head)
  per head pair, per q-chunk: scores^T = K_h Q_h^T  (row-packed K=64
  pairs), exp on ACT (scale=1/8; scores are O(0.2), no max needed),
  O^T_unnorm/denom = V_aug^T exp^T  ([65, q], denom = row 64),
  normalize via reciprocal + K=1 broadcast matmul + DVE multiply,
  spill O^T to DRAM; finally out = O^T^T woT re-loaded per t-tile.
Output: out [2048, 1024] partial (pre-bias) for this core's batch.
"""

import numpy as np
from contextlib import ExitStack

import concourse.bass as bass
import concourse.mybir as mybir
import concourse.tile as tile
from concourse import bacc
from concourse.bass_utils import run_bass_kernel_spmd

F32 = mybir.dt.float32
F32R = mybir.dt.float32r
AF = mybir.ActivationFunctionType

T = 2048          # tokens
D = 1024          # d_model
HK = 512          # 8 local heads x 64
NH = 8            # local heads
DH = 64           # head dim
NDT = 8           # d-tiles of 128
NTT = 16          # t-tiles of 128
NMT = 4           # (h,k) m-tiles of 128
NQC = 4           # q-chunks of 512
NST = 16          # s-tiles of 128
VW = NH * (DH + 1)  # V_aug width: 8 heads x (64 + ones col)


def build():
    nc = bacc.Bacc("TRN2", target_bir_lowering=False, debug=False)

    xT_d = nc.dram_tensor("xT", [D, T], F32R, kind="ExternalInput").ap()
    wq_d = nc.dram_tensor("wqT", [D, HK], F32R, kind="ExternalInput").ap()
    wk_d = nc.dram_tensor("wkT", [D, HK], F32R, kind="ExternalInput").ap()
    wv_d = nc.dram_tensor("wvT", [D, HK], F32R, kind="ExternalInput").ap()
    wo_d = nc.dram_tensor("woT", [HK, D], F32R, kind="ExternalInput").ap()
    bq_d = nc.dram_tensor("bq", [128, 4], F32, kind="ExternalInput").ap()
    bk_d = nc.dram_tensor("bk", [128, 4], F32, kind="ExternalInput").ap()
    ones_d = nc.dram_tensor("ones", [128, DH], F32R, kind="ExternalInput").ap()
    out_d = nc.dram_tensor("out", [T, D], F32, kind="ExternalOutput").ap()

    with tile.TileContext(nc) as tc, ExitStack() as ctx:
        const = ctx.enter_context(tc.tile_pool(name="const", bufs=1))
        bq_sb = const.tile([128, 4], F32, tag="bq", name="bq")
        bk_sb = const.tile([128, 4], F32, tag="bk", name="bk")
        ones_sb = const.tile([128, DH], F32R, tag="ones", name="ones")
        nc.sync.dma_start(bq_sb[:], bq_d)
        nc.sync.dma_start(bk_sb[:], bk_d)
        nc.sync.dma_start(ones_sb[:], ones_d)

        persist = ctx.enter_context(tc.tile_pool(name="persist", bufs=1))
        QT = [persist.tile([128, T], F32R, tag=f"qt{m}", name=f"qt{m}")
              for m in range(NMT)]
        KT = [persist.tile([128, T], F32R, tag=f"kt{m}", name=f"kt{m}")
              for m in range(NMT)]
        V = [persist.tile([128, VW], F32R, tag=f"v{t}", name=f"v{t}")
             for t in range(NTT)]

        dram = ctx.enter_context(tc.tile_pool(name="dram", bufs=1, space="DRAM"))
        OT_d = [dram.tile([128, T], F32R, tag=f"otd{j}", name=f"otd{j}")
                for j in range(NMT)]

        # ---------------- QKV projections ----------------
        with tc.tile_pool(name="wpool", bufs=1) as wpool, \
             tc.tile_pool(name="xpool", bufs=2) as xpool, \
             tc.tile_pool(name="qkv_ps", bufs=4, space="PSUM") as qps:
            wq_sb = [wpool.tile([128, HK], F32R, tag=f"wq{i}", name=f"wq{i}")
                     for i in range(NDT)]
            wk_sb = [wpool.tile([128, HK], F32R, tag=f"wk{i}", name=f"wk{i}")
                     for i in range(NDT)]
            wv_sb = [wpool.tile([128, HK], F32R, tag=f"wv{i}", name=f"wv{i}")
                     for i in range(NDT)]
            for i in range(NDT):
                nc.sync.dma_start(wq_sb[i][:], wq_d[i * 128:(i + 1) * 128, :])
                nc.sync.dma_start(wk_sb[i][:], wk_d[i * 128:(i + 1) * 128, :])
                nc.sync.dma_start(wv_sb[i][:], wv_d[i * 128:(i + 1) * 128, :])

            for c in range(4):  # t-chunks of 512
                csl = slice(c * 512, (c + 1) * 512)
                xt = [xpool.tile([128, 512], F32R, tag=f"x{i}", name=f"x{i}")
                      for i in range(NDT)]
                for i in range(NDT):
                    nc.sync.dma_start(xt[i][:], xT_d[i * 128:(i + 1) * 128, csl])
                # Q^T and K^T m-tiles for this chunk
                for m in range(NMT):
                    msl = slice(m * 128, (m + 1) * 128)
                    ps = qps.tile([128, 512], F32, tag="ps", name="ps")
                    for i in range(NDT):
                        nc.tensor.matmul(ps[:], wq_sb[i][:, msl], xt[i][:],
                                         start=(i == 0), stop=(i == NDT - 1))
                    nc.vector.tensor_scalar_add(QT[m][:, csl], ps[:],
                                                bq_sb[:, m:m + 1])
                    ps = qps.tile([128, 512], F32, tag="ps", name="ps")
                    for i in range(NDT):
                        nc.tensor.matmul(ps[:], wk_sb[i][:, msl], xt[i][:],
                                         start=(i == 0), stop=(i == NDT - 1))
                    nc.vector.tensor_scalar_add(KT[m][:, csl], ps[:],
                                                bk_sb[:, m:m + 1])
                # V t-tiles for this chunk (natural [t, (h,k)] layout + ones)
                for vt in range(4):
                    t_idx = c * 4 + vt
                    vsl = slice(vt * 128, (vt + 1) * 128)
                    ps = qps.tile([128, 512], F32, tag="ps", name="ps")
                    for i in range(NDT):
                        nc.tensor.matmul(ps[:], xt[i][:, vsl], wv_sb[i][:],
                                         start=(i == 0), stop=(i == NDT - 1))
                    v3 = V[t_idx][:].rearrange("p (h c) -> p h c", c=DH + 1)
                    nc.vector.tensor_copy(
                        v3[:, :, 0:DH], ps[:].rearrange("p (h c) -> p h c", c=DH))
                    nc.vector.tensor_copy(
                        v3[:, :, DH:DH + 1],
                        ones_sb[:, 0:NH].rearrange("p (h o) -> p h o", o=1))

        # ---------------- attention (per head pair) ----------------
        # scores^T for (st,st+1) land in one 2-bank psum tile so exp runs
        # at N=1024; the two heads of a pair are emitted adjacently so
        # their K=64 matmuls run concurrently on separate PE row groups.
        # AV consumes each exp tile right away (flash style), accumulating
        # O^T/denom in a [65, 512] psum per head.  The softmax reciprocal
        # is reshaped to [128, 4] via DMA (DVE reciprocal is serial per
        # lane), broadcast across partitions with a K=1 matmul, and
        # multiplied in on DVE; O^T spills to DRAM for the projection.
        with tc.tile_pool(name="epool", bufs=1) as epool, \
             tc.tile_pool(name="sc_ps", bufs=2, space="PSUM") as scps, \
             tc.tile_pool(name="av_ps", bufs=2, space="PSUM") as avps, \
             tc.tile_pool(name="opool", bufs=3) as opool:
            for j in range(NMT):  # head pair j: heads 2j (rows 0-63), 2j+1
                for qc in range(NQC):
                    qsl = slice(qc * 512, (qc + 1) * 512)
                    avp = {}
                    for hl in (0, 1):
                        avp[hl] = avps.tile([DH + 1, 512], F32,
                                            tag=f"av{hl}", name=f"av{hl}")
                    for sp in range(NST // 2):  # s-tile pairs
                        sc = {}
                        for hl in (0, 1):
                            sc[hl] = scps.tile([128, 1024], F32,
                                               tag="sc", name="sc")
                        for k in (0, 1):
                            st = 2 * sp + k
                            ssl = slice(st * 128, (st + 1) * 128)
                            for hl in (0, 1):
                                psl = slice(hl * 64, (hl + 1) * 64)
                                nc.tensor.matmul(
                                    sc[hl][:, k * 512:(k + 1) * 512],
                                    KT[j][psl, ssl], QT[j][psl, qsl])
                        es = {}
                        for hl in (0, 1):
                            e = epool.tile([128, 1024], F32R,
                                           tag=f"e{hl}_{sp}", name=f"e{hl}_{sp}")
                            nc.scalar.activation(e[:], sc[hl][:], AF.Exp,
                                                 scale=0.125)
                            es[hl] = e
                        for hl in (0, 1):
                            h = 2 * j + hl
                            for k in (0, 1):
                                st = 2 * sp + k
                                nc.tensor.matmul(
                                    avp[hl][:],
                                    V[st][:, h * 65:h * 65 + 65],
                                    es[hl][:, k * 512:(k + 1) * 512],
                                    start=(st == 0), stop=(st == NST - 1))
                    for hl in (0, 1):
                        # reshape denom row to [128, 4] so the reciprocal
                        # runs on all lanes, then back to a [1, 512] row
                        dnr = opool.tile([1, 512], F32, tag="dnr", name="dnr")
                        nc.vector.tensor_copy(dnr[:], avp[hl][DH:DH + 1, :])
                        dn4 = opool.tile([128, 4], F32, tag="dn4", name="dn4")
                        nc.sync.dma_start(dn4[:], dnr[:])
                        rc4 = opool.tile([128, 4], F32R, tag="rc4", name="rc4")
                        with nc.allow_low_precision(reason="fp32r softmax recip"):
                            nc.vector.reciprocal(rc4[:], dn4[:])
                        rcp = opool.tile([1, 512], F32R, tag="rcp", name="rcp")
                        nc.sync.dma_start(rcp[:], rc4[:])
                        bc = avps.tile([DH, 512], F32, tag=f"av{hl}",
                                       name="bc")
                        nc.tensor.matmul(bc[:], ones_sb[0:1, 0:DH], rcp[:])
                        bcs = opool.tile([DH, 512], F32R, tag="bcs", name="bcs")
                        nc.vector.tensor_copy(bcs[:], bc[:])
                        ot = opool.tile([DH, 512], F32R, tag="ot", name="ot")
                        nc.vector.tensor_mul(ot[:], avp[hl][0:DH, :], bcs[:])
                        nc.sync.dma_start(
                            OT_d[j][hl * 64:(hl + 1) * 64, qsl], ot[:])

        # ---------------- output projection ----------------
        with tc.tile_pool(name="fwp", bufs=1) as fwp, \
             tc.tile_pool(name="fop", bufs=1) as fop, \
             tc.tile_pool(name="fps", bufs=3, space="PSUM") as fps, \
             tc.tile_pool(name="foutp", bufs=3) as foutp:
            wo_sb = [fwp.tile([128, D], F32R, tag=f"wo{jj}", name=f"wo{jj}")
                     for jj in range(NMT)]
            OT_sb = [fop.tile([128, T], F32R, tag=f"otsb{jj}", name=f"otsb{jj}")
                     for jj in range(NMT)]
            for jj in range(NMT):
                nc.sync.dma_start(wo_sb[jj][:], wo_d[jj * 128:(jj + 1) * 128, :])
                nc.sync.dma_start(OT_sb[jj][:], OT_d[jj][:])
            for t in range(NTT):
                tsl = slice(t * 128, (t + 1) * 128)
                for dc in range(2):
                    dsl = slice(dc * 512, (dc + 1) * 512)
                    ps = fps.tile([128, 512], F32, tag="fp", name="fp")
                    for jj in range(NMT):
                        nc.tensor.matmul(ps[:], OT_sb[jj][:, tsl],
                                         wo_sb[jj][:, dsl],
                                         start=(jj == 0), stop=(jj == NMT - 1))
                    ob = foutp.tile([128, 512], F32, tag="ob", name="ob")
                    nc.vector.tensor_copy(ob[:], ps[:])
                    nc.sync.dma_start(out_d[tsl, dsl], ob[:])

    nc.compile()
    return nc


_NC_CACHE = None


def _get_nc():
    global _NC_CACHE
    if _NC_CACHE is None:
        _NC_CACHE = build()
    return _NC_CACHE


def _round_f32r(x):
    b = np.ascontiguousarray(x, dtype=np.float32).view(np.uint32)
    r = (b + 0x7FF + ((b >> 12) & 1)) & np.uint32(0xFFFFF000)
    return r.view(np.float32)


def _prep_core(x, W_Q, b_Q, W_K, b_K, W_V, b_V, W_O, core):
    b = core // 2
    hs = slice(8 * (core % 2), 8 * (core % 2) + 8)
    f32 = np.float32

    def bias_layout(bx):
        return np.ascontiguousarray(bx[hs].reshape(4, 128).T, dtype=f32)

    return {
        "xT": _round_f32r(x[b].T),
        "wqT": _round_f32r(W_Q[hs].reshape(HK, D).T),
        "wkT": _round_f32r(W_K[hs].reshape(HK, D).T),
        "wvT": _round_f32r(W_V[hs].reshape(HK, D).T),
        "woT": _round_f32r(W_O[hs].transpose(0, 2, 1).reshape(HK, D)),
        "bq": bias_layout(b_Q),
        "bk": bias_layout(b_K),
        "ones": np.ones((128, DH), dtype=f32),
    }


def kernel(x, W_Q, b_Q, W_K, b_K, W_V, b_V, W_O, b_O, _trace=False):
    nc = _get_nc()
    in_maps = [
        _prep_core(x, W_Q, b_Q, W_K, b_K, W_V, b_V, W_O, c) for c in range(8)
    ]
    res = run_bass_kernel_spmd(nc, in_maps, core_ids=list(range(8)),
                               trace=_trace)
    out = np.empty((4, T, D), dtype=np.float32)
    for b in range(4):
        # b_V enters additively after softmax (rows sum to 1): fold
        # b_V @ W_O per half-head shard into the host-side bias.
        acc = res.results[2 * b]["out"].astype(np.float32).copy()
        acc += res.results[2 * b + 1]["out"]
        bias = b_O.astype(np.float64).copy()
        for c in (2 * b, 2 * b + 1):
            hs = slice(8 * (c % 2), 8 * (c % 2) + 8)
            bias += np.einsum("hk,hdk->d", b_V[hs].astype(np.float64),
                              W_O[hs].astype(np.float64))
        out[b] = acc + bias.astype(np.float32)[None, :]
    if _trace:
        kernel.last_results = res
    return out


# revision 9
# speedup vs baseline: 2.1198x; 1.0474x over previous
"""Multi-head attention on 8 Trainium2 NeuronCores.

Problem shape: x[4, 2048, 1024], H=16 heads, Dh=64, fp32.
Sharding: core c handles batch b = c//2 and heads 8*(c%2) .. 8*(c%2)+8.
Each core computes its 8 heads' attention + the partial W_O contraction
for its batch; the host sums the two half-head partials per batch and
adds b_O (plus the b_V @ W_O constant row, folded host-side since
softmax rows sum to 1).  No collectives needed.

All matmuls run in float32r (fp32 storage, PE rounds to 12-bit
mantissa, 4x the fp32 rate at free-dim >= 256).  Host pre-rounds the
DRAM inputs to fp32r (RNE at 12 low mantissa bits) so DMA-loaded
operands satisfy the verifier's "rounded to FP32r" rule; on-chip
producers (ACT/DVE evictions) write float32r-typed tiles.

Device-side layout (per core, all host-pre-transposed so the kernel
never transposes anything):
  xT   [1024, 2048]  = x[b].T                                 [d, t]
  wqT/wkT/wvT [1024, 512] = W[heads].reshape(512,1024).T      [d, (h,k)]
  woT  [512, 1024]   = W_O[heads].transpose(0,2,1).reshape    [(h,k), d]
  bq/bk [128, 4]     per-partition bias layout (col m = (h,k) m*128..)
Pipeline per core:
  Q^T,K^T = W^T x^T  (+bias via ACT eviction)      [(h,k), t]
  V       = x W^T    ([t, 8*(64+1)] with a ones column per head)
  per head pair, per q-chunk: scores^T = K_h Q_h^T  (row-packed K=64
  pairs), exp on ACT (scale=1/8; scores are O(0.2), no max needed),
  O^T_unnorm/denom = V_aug^T exp^T  ([65, q], denom = row 64),
  normalize via reciprocal + K=1 broadcast matmul + DVE multiply,
  spill O^T to DRAM; finally out = O^T^T woT re-loaded per t-tile.
Output: out [2048, 1024] partial (pre-bias) for this core's batch.
"""

import numpy as np
from contextlib import ExitStack

import concourse.bass as bass
import concourse.mybir as mybir
import concourse.tile as tile
from concourse import bacc
from concourse.bass_utils import run_bass_kernel_spmd

F32 = mybir.dt.float32
F32R = mybir.dt.float32r
AF = mybir.ActivationFunctionType

T = 2048          # tokens
D = 1024          # d_model
HK = 512          # 8 local heads x 64
NH = 8            # local heads
DH = 64           # head dim
NDT = 8           # d-tiles of 128
NTT = 16          # t-tiles of 128
NMT = 4           # (h,k) m-tiles of 128
NQC = 4           # q-chunks of 512
NST = 16          # s-tiles of 128
VW = NH * (DH + 1)  # V_aug width: 8 heads x (64 + ones col)


def build():
    nc = bacc.Bacc("TRN2", target_bir_lowering=False, debug=False)

    xT_d = nc.dram_tensor("xT", [D, T], F32R, kind="ExternalInput").ap()
    wq_d = nc.dram_tensor("wqT", [D, HK], F32R, kind="ExternalInput").ap()
    wk_d = nc.dram_tensor("wkT", [D, HK], F32R, kind="ExternalInput").ap()
    wv_d = nc.dram_tensor("wvT", [D, HK], F32R, kind="ExternalInput").ap()
    wo_d = nc.dram_tensor("woT", [HK, D], F32R, kind="ExternalInput").ap()
    bq_d = nc.dram_tensor("bq", [128, 4], F32, kind="ExternalInput").ap()
    bk_d = nc.dram_tensor("bk", [128, 4], F32, kind="ExternalInput").ap()
    ones_d = nc.dram_tensor("ones", [128, DH], F32R, kind="ExternalInput").ap()
    out_d = nc.dram_tensor("out", [T, D], F32, kind="ExternalOutput").ap()

    with tile.TileContext(nc) as tc, ExitStack() as ctx:
        const = ctx.enter_context(tc.tile_pool(name="const", bufs=1))
        bq_sb = const.tile([128, 4], F32, tag="bq", name="bq")
        bk_sb = const.tile([128, 4], F32, tag="bk", name="bk")
        ones_sb = const.tile([128, DH], F32R, tag="ones", name="ones")
        nc.sync.dma_start(bq_sb[:], bq_d)
        nc.sync.dma_start(bk_sb[:], bk_d)
        nc.sync.dma_start(ones_sb[:], ones_d)

        persist = ctx.enter_context(tc.tile_pool(name="persist", bufs=1))
        QT = [persist.tile([128, T], F32R, tag=f"qt{m}", name=f"qt{m}")
              for m in range(NMT)]
        KT = [persist.tile([128, T], F32R, tag=f"kt{m}", name=f"kt{m}")
              for m in range(NMT)]
        V = [persist.tile([128, VW], F32R, tag=f"v{t}", name=f"v{t}")
             for t in range(NTT)]


        # ---------------- QKV projections ----------------
        with tc.tile_pool(name="wpool", bufs=1) as wpool, \
             tc.tile_pool(name="xpool", bufs=2) as xpool, \
             tc.tile_pool(name="qkv_ps", bufs=4, space="PSUM") as qps:
            wq_sb = [wpool.tile([128, HK], F32R, tag=f"wq{i}", name=f"wq{i}")
                     for i in range(NDT)]
            wk_sb = [wpool.tile([128, HK], F32R, tag=f"wk{i}", name=f"wk{i}")
                     for i in range(NDT)]
            wv_sb = [wpool.tile([128, HK], F32R, tag=f"wv{i}", name=f"wv{i}")
                     for i in range(NDT)]
            for i in range(NDT):
                nc.sync.dma_start(wq_sb[i][:], wq_d[i * 128:(i + 1) * 128, :])
                nc.sync.dma_start(wk_sb[i][:], wk_d[i * 128:(i + 1) * 128, :])
                nc.sync.dma_start(wv_sb[i][:], wv_d[i * 128:(i + 1) * 128, :])

            for c in range(4):  # t-chunks of 512
                csl = slice(c * 512, (c + 1) * 512)
                xt = [xpool.tile([128, 512], F32R, tag=f"x{i}", name=f"x{i}")
                      for i in range(NDT)]
                for i in range(NDT):
                    nc.sync.dma_start(xt[i][:], xT_d[i * 128:(i + 1) * 128, csl])
                # Q^T and K^T m-tiles for this chunk
                for m in range(NMT):
                    msl = slice(m * 128, (m + 1) * 128)
                    ps = qps.tile([128, 512], F32, tag="ps", name="ps")
                    for i in range(NDT):
                        nc.tensor.matmul(ps[:], wq_sb[i][:, msl], xt[i][:],
                                         start=(i == 0), stop=(i == NDT - 1))
                    nc.vector.tensor_scalar_add(QT[m][:, csl], ps[:],
                                                bq_sb[:, m:m + 1])
                    ps = qps.tile([128, 512], F32, tag="ps", name="ps")
                    for i in range(NDT):
                        nc.tensor.matmul(ps[:], wk_sb[i][:, msl], xt[i][:],
                                         start=(i == 0), stop=(i == NDT - 1))
                    nc.vector.tensor_scalar_add(KT[m][:, csl], ps[:],
                                                bk_sb[:, m:m + 1])
                # V t-tiles for this chunk (natural [t, (h,k)] layout + ones)
                for vt in range(4):
                    t_idx = c * 4 + vt
                    vsl = slice(vt * 128, (vt + 1) * 128)
                    ps = qps.tile([128, 512], F32, tag="ps", name="ps")
                    for i in range(NDT):
                        nc.tensor.matmul(ps[:], xt[i][:, vsl], wv_sb[i][:],
                                         start=(i == 0), stop=(i == NDT - 1))
                    v3 = V[t_idx][:].rearrange("p (h c) -> p h c", c=DH + 1)
                    nc.vector.tensor_copy(
                        v3[:, :, 0:DH], ps[:].rearrange("p (h c) -> p h c", c=DH))
                    nc.vector.tensor_copy(
                        v3[:, :, DH:DH + 1],
                        ones_sb[:, 0:NH].rearrange("p (h o) -> p h o", o=1))

        # ---------------- attention + projection, per q-chunk ----------------
        # scores^T for (st,st+1) land in one 2-bank psum tile so exp runs
        # at N=1024; the two heads of a pair are emitted adjacently so
        # their K=64 matmuls run concurrently on separate PE row groups.
        # AV consumes each exp tile right away (flash style), accumulating
        # O^T/denom in a [65, 512] psum per head, which is copied to SBUF
        # immediately so the psum slot frees for the next chunk; the
        # softmax reciprocal runs in a [128, 4] reshape (DVE reciprocal is
        # serial per lane), is broadcast across partitions with a K=1
        # matmul, and multiplied in on DVE.  O^T stays in SBUF and the
        # output projection for each q-chunk follows immediately.
        with tc.tile_pool(name="epool", bufs=1) as epool, \
             tc.tile_pool(name="otpool", bufs=2) as otpool, \
             tc.tile_pool(name="fwp", bufs=1) as fwp, \
             tc.tile_pool(name="sc_ps", bufs=2, space="PSUM") as scps, \
             tc.tile_pool(name="av_ps", bufs=1, space="PSUM") as avps, \
             tc.tile_pool(name="bc_ps", bufs=1, space="PSUM") as bcps, \
             tc.tile_pool(name="fps", bufs=1, space="PSUM") as fps, \
             tc.tile_pool(name="opool", bufs=3) as opool, \
             tc.tile_pool(name="foutp", bufs=3) as foutp:
            wo_sb = [fwp.tile([128, D], F32R, tag=f"wo{jj}", name=f"wo{jj}")
                     for jj in range(NMT)]
            for jj in range(NMT):
                nc.sync.dma_start(wo_sb[jj][:], wo_d[jj * 128:(jj + 1) * 128, :])
            for qc in range(NQC):
                qsl = slice(qc * 512, (qc + 1) * 512)
                OT = [otpool.tile([128, 512], F32R, tag=f"ot{j}", name=f"ot{j}")
                      for j in range(NMT)]
                for j in range(NMT):  # head pair j: heads 2j, 2j+1
                    avp = {}
                    for hl in (0, 1):
                        avp[hl] = avps.tile([DH + 1, 512], F32,
                                            tag=f"av{hl}", name=f"av{hl}")
                    for sp in range(NST // 2):  # s-tile pairs
                        sc = {}
                        for hl in (0, 1):
                            sc[hl] = scps.tile([128, 1024], F32,
                                               tag="sc", name="sc")
                        for k in (0, 1):
                            st = 2 * sp + k
                            ssl = slice(st * 128, (st + 1) * 128)
                            for hl in (0, 1):
                                psl = slice(hl * 64, (hl + 1) * 64)
                                nc.tensor.matmul(
                                    sc[hl][:, k * 512:(k + 1) * 512],
                                    KT[j][psl, ssl], QT[j][psl, qsl])
                        es = {}
                        for hl in (0, 1):
                            e = epool.tile([128, 1024], F32R,
                                           tag=f"e{hl}_{sp % 4}",
                                           name=f"e{hl}_{sp % 4}")
                            nc.scalar.activation(e[:], sc[hl][:], AF.Exp,
                                                 scale=0.125)
                            es[hl] = e
                        for hl in (0, 1):
                            h = 2 * j + hl
                            for k in (0, 1):
                                st = 2 * sp + k
                                nc.tensor.matmul(
                                    avp[hl][:],
                                    V[st][:, h * 65:h * 65 + 65],
                                    es[hl][:, k * 512:(k + 1) * 512],
                                    start=(st == 0), stop=(st == NST - 1))
                    for hl in (0, 1):
                        # copy accumulator out of PSUM right away so the
                        # slot frees for the next chunk's AV matmuls
                        avs = opool.tile([DH + 1, 512], F32, tag="avs",
                                         name="avs")
                        nc.vector.tensor_copy(avs[:], avp[hl][:])
                        dn4 = opool.tile([128, 4], F32, tag="dn4", name="dn4")
                        nc.sync.dma_start(dn4[:], avs[DH:DH + 1, :])
                        rc4 = opool.tile([128, 4], F32R, tag="rc4", name="rc4")
                        with nc.allow_low_precision(reason="fp32r softmax recip"):
                            nc.vector.reciprocal(rc4[:], dn4[:])
                        rcp = opool.tile([1, 512], F32R, tag="rcp", name="rcp")
                        nc.sync.dma_start(rcp[:], rc4[:])
                        bc = bcps.tile([DH, 512], F32, tag="bc", name="bc")
                        nc.tensor.matmul(bc[:], ones_sb[0:1, 0:DH], rcp[:])
                        bcs = opool.tile([DH, 512], F32R, tag="bcs", name="bcs")
                        nc.vector.tensor_copy(bcs[:], bc[:])
                        nc.vector.tensor_mul(OT[j][hl * 64:(hl + 1) * 64, :],
                                             avs[0:DH, :], bcs[:])
                # ---- output projection for this q-chunk ----
                for tt in range(4):
                    tq = qc * 512 + tt * 128
                    for dc in range(2):
                        dsl = slice(dc * 512, (dc + 1) * 512)
                        ps = fps.tile([128, 512], F32, tag="fp", name="fp")
                        for jj in range(NMT):
                            nc.tensor.matmul(ps[:],
                                             OT[jj][:, tt * 128:(tt + 1) * 128],
                                             wo_sb[jj][:, dsl],
                                             start=(jj == 0),
                                             stop=(jj == NMT - 1))
                        ob = foutp.tile([128, 512], F32, tag="ob", name="ob")
                        nc.vector.tensor_copy(ob[:], ps[:])
                        nc.sync.dma_start(out_d[tq:tq + 128, dsl], ob[:])

    nc.compile()
    return nc


_NC_CACHE = None


def _get_nc():
    global _NC_CACHE
    if _NC_CACHE is None:
        _NC_CACHE = build()
    return _NC_CACHE


def _round_f32r(x):
    b = np.ascontiguousarray(x, dtype=np.float32).view(np.uint32)
    r = (b + 0x7FF + ((b >> 12) & 1)) & np.uint32(0xFFFFF000)
    return r.view(np.float32)


def _prep_core(x, W_Q, b_Q, W_K, b_K, W_V, b_V, W_O, core):
    b = core // 2
    hs = slice(8 * (core % 2), 8 * (core % 2) + 8)
    f32 = np.float32

    def bias_layout(bx):
        return np.ascontiguousarray(bx[hs].reshape(4, 128).T, dtype=f32)

    return {
        "xT": _round_f32r(x[b].T),
        "wqT": _round_f32r(W_Q[hs].reshape(HK, D).T),
        "wkT": _round_f32r(W_K[hs].reshape(HK, D).T),
        "wvT": _round_f32r(W_V[hs].reshape(HK, D).T),
        "woT": _round_f32r(W_O[hs].transpose(0, 2, 1).reshape(HK, D)),
        "bq": bias_layout(b_Q),
        "bk": bias_layout(b_K),
        "ones": np.ones((128, DH), dtype=f32),
    }


def kernel(x, W_Q, b_Q, W_K, b_K, W_V, b_V, W_O, b_O, _trace=False):
    nc = _get_nc()
    in_maps = [
        _prep_core(x, W_Q, b_Q, W_K, b_K, W_V, b_V, W_O, c) for c in range(8)
    ]
    res = run_bass_kernel_spmd(nc, in_maps, core_ids=list(range(8)),
                               trace=_trace)
    out = np.empty((4, T, D), dtype=np.float32)
    for b in range(4):
        # b_V enters additively after softmax (rows sum to 1): fold
        # b_V @ W_O per half-head shard into the host-side bias.
        acc = res.results[2 * b]["out"].astype(np.float32).copy()
        acc += res.results[2 * b + 1]["out"]
        bias = b_O.astype(np.float64).copy()
        for c in (2 * b, 2 * b + 1):
            hs = slice(8 * (c % 2), 8 * (c % 2) + 8)
            bias += np.einsum("hk,hdk->d", b_V[hs].astype(np.float64),
                              W_O[hs].astype(np.float64))
        out[b] = acc + bias.astype(np.float32)[None, :]
    if _trace:
        kernel.last_results = res
    return out
